# revision 33
# baseline (speedup 1.0000x reference)
"""Trainium2 kernel for nn_Direction: out = input @ Q.T, Q from QR(weight + 1e-8).

Strategy (LAYOUT="inv", the measured-best structure):
  - Host: QR of the small 512x512 weight (jax-on-CPU fp32), cast input + Q.T
    to fp16, pack each core's batch shard as a0[p, it, k, b] =
    A.T[k*128+p, it*2048+b] so every per-iteration input DMA is fully
    contiguous per partition.
  - Device (8 cores, data-parallel over batch): single fp16 pass
    (rel err 3.6e-4 vs the fp32 reference, vs the 2e-2 gate). Q-stationary
    ("inverted") matmul: for each 128-wide n-tile of Q, the stationary
    operand qt[k, n-slice] is reused by 4 consecutive matmuls streaming
    2048 batch cols (moving, N=512 each), amortizing the PE weight load —
    measured ~272 ns/MM vs ~304 ns/MM for the A-stationary form. 16-MM
    accumulation groups into 2x 2-bank PSUM tiles; evictions cast fp32->fp16
    on alternating vector/scalar engines; output written transposed+packed,
    one 2 MB store per iteration on the sync HWDGE ring (stores on the
    scalar ring head-of-line-block against evictions and measure ~15 us
    slower). Host unpacks/upcasts to fp32 and concatenates shards.

Measured (differential on-device rep loop, device-resident tensors):
~135-140 us/core for the full kernel, equal to the pure-MM-stream floor
(512 MMs x ~272 ns); input(16 MB)+output(16 MB) DMA and PSUM evictions are
fully overlapped. fp16 streaming bound is 512x512cols/2.4GHz = 110 us; the
~270 ns/MM sustained rate is consistent with the P0 sustained-power PE
clock (~2.0 GHz) plus issue overhead. 3-pass fp16x2 baseline measured
417976 ns by the same harness.
"""

import numpy as np

import concourse.bacc as bacc
import concourse.mybir as mybir
import concourse.tile as tile
from concourse.bass_utils import run_bass_kernel_spmd

B_FULL = 131072
D = 512
N_CORES = 8
B_LOC = B_FULL // N_CORES  # 16384
P = 128
BT = 512  # batch rows per loop iteration
KT = D // P  # 4 k-tiles
SB = BT // P  # 4 psum sub-tiles per iteration

# precision mode: "fp32" | "fp16" | "bf16" | "fp16x2"
MODE = "fp16"
# kernel structure: "plain" | "packed" | "inv"  (see the _build_* docstrings)
LAYOUT = "inv"
BT_TILE = 2048  # batch rows per loop iteration (packed/inv layouts)
BUILD_KW = dict(pchunk=2, ps_bufs=4, ain_bufs=4, aout_bufs=4, evict="alt")

_DT = {
    "fp32": mybir.dt.float32,
    "fp16": mybir.dt.float16,
    "bf16": mybir.dt.bfloat16,
    "fp16x2": mybir.dt.float16,
}

# (a_input, q_input) matmul passes, accumulated in PSUM.
_PASSES = {
    "fp32": [("a0", "q0")],
    "fp16": [("a0", "q0")],
    "bf16": [("a0", "q0")],
    "fp16x2": [("a0", "q0"), ("a1", "q0"), ("a0", "q1")],
}

_CACHE = {}


def _build(mode, b_loc, reps=1, dynamic=False, bt=BT, ain_bufs=4, aout_bufs=4,
           ps_bufs=8, evict="alt", out16=True, do_in=True, do_out=True,
           out_q="sync", groups_mult=1, no_evict=False, bench_internal=False,
           samew=False):
    dt_in = _DT[mode]
    dt_out = mybir.dt.float16 if out16 else mybir.dt.float32
    passes = _PASSES[mode]
    a_names = sorted({a for a, _ in passes})
    q_names = sorted({q for _, q in passes})
    n_iter = b_loc // bt
    sb_n = bt // P

    nc = bacc.Bacc("TRN2", target_bir_lowering=False, debug=False)
    kin = "Internal" if bench_internal else "ExternalInput"
    kout = "Internal" if bench_internal else "ExternalOutput"
    a_dram = {
        n: nc.dram_tensor(n, [D, b_loc], dt_in, kind=kin).ap()
        for n in a_names
    }
    q_dram = {
        n: nc.dram_tensor(n, [D, D], dt_in, kind=kin).ap()
        for n in q_names
    }
    out_dram = nc.dram_tensor("out", [b_loc, D], dt_out, kind=kout).ap()
    if bench_internal:
        seed_dram = nc.dram_tensor(
            "seed", [1, 64], mybir.dt.int32, kind="ExternalInput"
        ).ap()
        dout_dram = nc.dram_tensor(
            "dout", [1, 64], mybir.dt.int32, kind="ExternalOutput"
        ).ap()

    with tile.TileContext(nc) as tc:
        with (
            tc.tile_pool(name="consts", bufs=1) as consts,
            tc.tile_pool(name="ain", bufs=ain_bufs) as ain,
            tc.tile_pool(name="aout", bufs=aout_bufs) as aout,
            tc.tile_pool(name="ps", bufs=ps_bufs, space="PSUM") as ps_pool,
        ):
            q_tiles = {}
            for qn in q_names:
                qt = consts.tile([P, KT, D], dt_in, name=f"qt_{qn}")
                nc.sync.dma_start(
                    out=qt[:, :, :],
                    in_=q_dram[qn].rearrange("(k p) n -> p k n", p=P),
                )
                q_tiles[qn] = qt

            a_const = {}
            if not do_in:
                for an in a_names:
                    at = consts.tile([P, KT, bt], dt_in, name=f"ac_{an}")
                    src = a_dram[an].rearrange("(k p) b -> p k b", p=P)[:, :, 0:bt]
                    nc.sync.dma_start(out=at[:, :, :], in_=src)
                    a_const[an] = at

            out_eng = {"sync": nc.sync, "scalar": nc.scalar}[out_q]

            def body():
                for it in range(n_iter):
                    a_tiles = {}
                    for an in a_names:
                        if not do_in:
                            a_tiles[an] = a_const[an]
                            continue
                        at = ain.tile(
                            [P, KT, bt], dt_in, name=f"at_{an}", tag=f"at_{an}"
                        )
                        src = a_dram[an].rearrange("(k p) b -> p k b", p=P)[
                            :, :, it * bt : (it + 1) * bt
                        ]
                        nc.sync.dma_start(out=at[:, :, :], in_=src)
                        a_tiles[an] = at
                    for sb in range(sb_n):
                        ps = ps_pool.tile(
                            [P, D], mybir.dt.float32, name="ps", tag="ps"
                        )
                        n_mm = len(passes) * KT * groups_mult
                        mm = 0
                        for _g in range(groups_mult):
                            for an, qn in passes:
                                at = a_tiles[an]
                                qt = q_tiles[qn]
                                for k in range(KT):
                                    stat = (
                                        at[:, 0, 0:P]
                                        if samew
                                        else at[:, k, sb * P : (sb + 1) * P]
                                    )
                                    nc.tensor.matmul(
                                        ps[:, :],
                                        stat,
                                        qt[:, k, :],
                                        start=(mm == 0),
                                        stop=(mm == n_mm - 1),
                                    )
                                    mm += 1
                        if no_evict:
                            continue
                        ot = aout.tile([P, D], dt_out, name="ot", tag="ot")
                        if evict == "any":
                            nc.any.tensor_copy(ot[:, :], ps[:, :])
                        elif evict == "vector":
                            nc.vector.tensor_copy(ot[:, :], ps[:, :])
                        elif evict == "alt":
                            if sb % 2 == 0:
                                nc.vector.tensor_copy(ot[:, :], ps[:, :])
                            else:
                                nc.scalar.activation(
                                    ot[:, :],
                                    ps[:, :],
                                    mybir.ActivationFunctionType.Copy,
                                )
                        b0 = it * bt + sb * P
                        if do_out:
                            out_eng.dma_start(
                                out=out_dram[b0 : b0 + P, :], in_=ot[:, :]
                            )

            if bench_internal:
                st = consts.tile([1, 64], mybir.dt.int32, name="seed_t")
                nc.sync.dma_start(out=st[:, :], in_=seed_dram[:, :])
                nc.sync.dma_start(out=dout_dram[:, :], in_=st[:, :])

            if dynamic == "unroll" and reps > 1:
                tc.For_i_unrolled(0, reps, 1, lambda iv: body(), max_unroll=4)
            elif dynamic and reps > 1:
                with tc.For_i(0, reps, 1):
                    body()
            else:
                for _ in range(reps):
                    body()

    nc.compile()
    return nc


def _build_packed(mode, b_loc, reps=1, dynamic=False, bt=2048, ain_bufs=3,
                  aout_bufs=3, ps_bufs=8, evict="alt", out_q="scalar",
                  bench_internal=False, pb=1, do_in=True, do_out=True,
                  no_evict=False):
    """Single-pass matmul with host-packed DRAM layouts.

    a_dram[p, it, k, b] = A.T[k*128+p, it*bt+b]  (fp16/bf16) — each per-iter
    input DMA reads KT*bt*2 bytes fully contiguous per partition.
    out_dram[p, it, s, n] = out[it*bt + s*128 + p, n] (fp16) — each per-iter
    output DMA writes sb_n*D*2 bytes fully contiguous per partition; the host
    unpermutes. One dma_start each way per iteration; outputs go on the
    scalar HWDGE ring so stores never head-of-line-block input loads on the
    sync ring.
    """
    dt_in = _DT[mode]
    dt_out = mybir.dt.float16
    assert len(_PASSES[mode]) == 1, "packed build supports single-pass modes"
    n_iter = b_loc // bt
    sb_n = bt // P

    nc = bacc.Bacc("TRN2", target_bir_lowering=False, debug=False)
    kin = "Internal" if bench_internal else "ExternalInput"
    kout = "Internal" if bench_internal else "ExternalOutput"
    a_dram = nc.dram_tensor("a0", [P, n_iter, KT, bt], dt_in, kind=kin).ap()
    q_dram = nc.dram_tensor("q0", [D, D], dt_in, kind=kin).ap()
    out_dram = nc.dram_tensor(
        "out", [P, n_iter, sb_n, D], dt_out, kind=kout
    ).ap()
    if bench_internal:
        seed_dram = nc.dram_tensor(
            "seed", [1, 64], mybir.dt.int32, kind="ExternalInput"
        ).ap()
        dout_dram = nc.dram_tensor(
            "dout", [1, 64], mybir.dt.int32, kind="ExternalOutput"
        ).ap()

    with tile.TileContext(nc) as tc:
        with (
            tc.tile_pool(name="consts", bufs=1) as consts,
            tc.tile_pool(name="ain", bufs=ain_bufs) as ain,
            tc.tile_pool(name="aout", bufs=aout_bufs) as aout,
            tc.tile_pool(name="ps", bufs=ps_bufs, space="PSUM") as ps_pool,
        ):
            qt = consts.tile([P, KT, D], dt_in, name="qt")
            nc.sync.dma_start(
                out=qt[:, :, :],
                in_=q_dram.rearrange("(k p) n -> p k n", p=P),
            )
            out_eng = {"sync": nc.sync, "scalar": nc.scalar}[out_q]

            a_res = None
            if not do_in:
                a_res = consts.tile([P, KT, bt], dt_in, name="a_res")
                nc.sync.dma_start(out=a_res[:, :, :], in_=a_dram[:, 0, :, :])

            assert sb_n % pb == 0 and pb * ps_bufs <= 8

            def body():
                for it in range(n_iter):
                    if do_in:
                        at = ain.tile([P, KT, bt], dt_in, name="at", tag="at")
                        nc.sync.dma_start(
                            out=at[:, :, :], in_=a_dram[:, it, :, :]
                        )
                    else:
                        at = a_res
                    ot = aout.tile([P, sb_n, D], dt_out, name="ot", tag="ot")
                    for g in range(sb_n // pb):
                        ps = ps_pool.tile(
                            [P, pb, D], mybir.dt.float32, name="ps", tag="ps"
                        )
                        for j in range(pb):
                            sb = g * pb + j
                            for k in range(KT):
                                nc.tensor.matmul(
                                    ps[:, j, :],
                                    at[:, k, sb * P : (sb + 1) * P],
                                    qt[:, k, :],
                                    start=(k == 0),
                                    stop=(k == KT - 1),
                                )
                        if no_evict:
                            continue
                        dst = ot[:, g * pb : (g + 1) * pb, :]
                        if evict == "alt" and g % 2 == 1:
                            nc.scalar.activation(
                                dst,
                                ps[:, :, :],
                                mybir.ActivationFunctionType.Copy,
                            )
                        elif evict == "scalar":
                            nc.scalar.activation(
                                dst,
                                ps[:, :, :],
                                mybir.ActivationFunctionType.Copy,
                            )
                        else:
                            nc.vector.tensor_copy(dst, ps[:, :, :])
                    if do_out and not no_evict:
                        out_eng.dma_start(
                            out=out_dram[:, it, :, :], in_=ot[:, :, :]
                        )

            if bench_internal:
                st = consts.tile([1, 64], mybir.dt.int32, name="seed_t")
                nc.sync.dma_start(out=st[:, :], in_=seed_dram[:, :])
                nc.sync.dma_start(out=dout_dram[:, :], in_=st[:, :])

            if dynamic and reps > 1:
                with tc.For_i(0, reps, 1):
                    body()
            else:
                for _ in range(reps):
                    body()

    nc.compile()
    return nc


def _build_inv(mode, b_loc, reps=1, dynamic=False, bt=2048, ain_bufs=3,
               aout_bufs=3, ps_bufs=2, evict="alt", out_q="sync",
               bench_internal=False, expldw=False, do_in=True, do_out=True,
               no_evict=False, pchunk=None):
    """Q-stationary inverted matmul: out.T tiles in PSUM.

    For each 128-wide n-tile of Q, the stationary operand qt[:, k, n-slice]
    is reused by `bt/512` consecutive matmuls streaming A chunks (moving,
    N=512 batch cols), amortizing the PE weight load. PSUM tile = [128(n),
    chunks, 512(b)] fp32 spanning `chunks` banks; 16-MM accumulation groups.
    Output lands transposed; host unpacks.

    a_dram[p, it, k, b] = A.T[k*128+p, it*bt+b] (same as packed layout).
    out_dram[p, n, it, b] = out[it*bt + b, n*128 + p].
    """
    dt_in = _DT[mode]
    dt_out = mybir.dt.float16
    assert len(_PASSES[mode]) == 1
    n_iter = b_loc // bt
    ch = bt // 512  # moving chunks per iteration
    nt = D // P  # 4 n-tiles
    pchunk = pchunk or ch  # chunks per PSUM tile (eviction granularity)
    assert ch % pchunk == 0
    assert pchunk * ps_bufs <= 8

    nc = bacc.Bacc("TRN2", target_bir_lowering=False, debug=False)
    kin = "Internal" if bench_internal else "ExternalInput"
    kout = "Internal" if bench_internal else "ExternalOutput"
    a_dram = nc.dram_tensor("a0", [P, n_iter, KT, bt], dt_in, kind=kin).ap()
    q_dram = nc.dram_tensor("q0", [D, D], dt_in, kind=kin).ap()
    out_dram = nc.dram_tensor(
        "out", [P, nt, n_iter, bt], dt_out, kind=kout
    ).ap()
    if bench_internal:
        seed_dram = nc.dram_tensor(
            "seed", [1, 64], mybir.dt.int32, kind="ExternalInput"
        ).ap()
        dout_dram = nc.dram_tensor(
            "dout", [1, 64], mybir.dt.int32, kind="ExternalOutput"
        ).ap()

    with tile.TileContext(nc) as tc:
        with (
            tc.tile_pool(name="consts", bufs=1) as consts,
            tc.tile_pool(name="ain", bufs=ain_bufs) as ain,
            tc.tile_pool(name="aout", bufs=aout_bufs) as aout,
            tc.tile_pool(name="ps", bufs=ps_bufs, space="PSUM") as ps_pool,
        ):
            qt = consts.tile([P, KT, D], dt_in, name="qt")
            nc.sync.dma_start(
                out=qt[:, :, :],
                in_=q_dram.rearrange("(k p) n -> p k n", p=P),
            )
            out_eng = {"sync": nc.sync, "scalar": nc.scalar}[out_q]

            a_res = None
            if not do_in:
                a_res = consts.tile([P, KT, bt], dt_in, name="a_res")
                nc.sync.dma_start(out=a_res[:, :, :], in_=a_dram[:, 0, :, :])

            def body():
                for it in range(n_iter):
                    if do_in:
                        at = ain.tile([P, KT, bt], dt_in, name="at", tag="at")
                        nc.sync.dma_start(
                            out=at[:, :, :], in_=a_dram[:, it, :, :]
                        )
                    else:
                        at = a_res
                    ot = aout.tile([P, nt, bt], dt_out, name="ot", tag="ot")
                    n_ps = ch // pchunk
                    for n in range(nt):
                        pss = [
                            ps_pool.tile(
                                [P, pchunk, 512], mybir.dt.float32,
                                name="ps", tag="ps",
                            )
                            for _ in range(n_ps)
                        ]
                        for k in range(KT):
                            w = qt[:, k, n * P : (n + 1) * P]
                            if expldw:
                                nc.tensor.ldweights(w)
                            for c in range(ch):
                                mm = nc.tensor.matmul(
                                    pss[c // pchunk][:, c % pchunk, :],
                                    w,
                                    at[:, k, c * 512 : (c + 1) * 512],
                                    start=(k == 0),
                                    stop=(k == KT - 1),
                                )
                                if expldw:
                                    mm.ins.ldweights = False
                        if no_evict:
                            continue
                        for t in range(n_ps):
                            dst = ot[:, n, t * pchunk * 512 :
                                     (t + 1) * pchunk * 512]
                            if evict == "alt" and (n * n_ps + t) % 2 == 1:
                                nc.scalar.activation(
                                    dst,
                                    pss[t][:, :, :],
                                    mybir.ActivationFunctionType.Copy,
                                )
                            else:
                                nc.vector.tensor_copy(dst, pss[t][:, :, :])
                    if do_out and not no_evict:
                        out_eng.dma_start(
                            out=out_dram[:, :, it, :], in_=ot[:, :, :]
                        )

            if bench_internal:
                st = consts.tile([1, 64], mybir.dt.int32, name="seed_t")
                nc.sync.dma_start(out=st[:, :], in_=seed_dram[:, :])
                nc.sync.dma_start(out=dout_dram[:, :], in_=st[:, :])

            if dynamic and reps > 1:
                with tc.For_i(0, reps, 1):
                    body()
            else:
                for _ in range(reps):
                    body()

    nc.compile()
    return nc


def _unpack_out_inv(res, n_cores, b_loc, bt=2048):
    """out_dram[p, n, it, b] -> (n_cores*b_loc, D) fp32."""
    n_iter = b_loc // bt
    outs = []
    for i in range(n_cores):
        o = np.asarray(res.results[i]["out"])  # [P, nt, n_iter, bt]
        # out[it*bt + b, n*128 + p] = o[p, n, it, b]
        o = o.transpose(2, 3, 1, 0).reshape(b_loc, D)
        outs.append(o)
    return np.concatenate(outs, axis=0).astype(np.float32)


def _get_nc(mode, b_loc, **kw):
    return _get_nc_reps(mode, b_loc, 1, **kw)


def _get_nc_reps(mode, b_loc, reps, dynamic=False, layout="plain", **kw):
    key = (mode, b_loc, reps, dynamic, layout, tuple(sorted(kw.items())))
    if key not in _CACHE:
        builder = {
            "plain": _build,
            "packed": _build_packed,
            "inv": _build_inv,
        }[layout]
        _CACHE[key] = builder(mode, b_loc, reps, dynamic, **kw)
    return _CACHE[key]


def _split16(x):
    hi = x.astype(np.float16)
    lo = (x - hi.astype(np.float32)).astype(np.float16)
    return hi, lo


def _prep_inputs(mode, input_np, qt_np, n_cores, b_loc):
    """Build per-core input maps. input_np: (n_cores*b_loc, D) fp32 row-major.
    qt_np: (D, D) fp32, qt_np[m, n] = Q[n, m]."""
    maps = []
    if mode == "fp16x2":
        qh, ql = _split16(qt_np)
        for i in range(n_cores):
            at = np.ascontiguousarray(input_np[i * b_loc : (i + 1) * b_loc].T)
            ah, al = _split16(at)
            maps.append({"a0": ah, "a1": al, "q0": qh, "q1": ql})
    else:
        if mode == "bf16":
            import ml_dtypes

            cast_dt = ml_dtypes.bfloat16
        else:
            cast_dt = {"fp32": np.float32, "fp16": np.float16}[mode]
        q0 = qt_np.astype(cast_dt)
        # cast first (vectorized over the full row-major array), then
        # transpose per-core shards
        inp_c = input_np.astype(cast_dt)
        for i in range(n_cores):
            at = np.ascontiguousarray(inp_c[i * b_loc : (i + 1) * b_loc].T)
            maps.append({"a0": at, "q0": q0})
    return maps


def _prep_inputs_packed(mode, input_np, qt_np, n_cores, b_loc, bt=2048):
    """Packed per-core input maps: a0[p, it, k, b] = A.T[k*128+p, it*bt+b]."""
    if mode == "bf16":
        import ml_dtypes

        cast_dt = ml_dtypes.bfloat16
    else:
        cast_dt = {"fp32": np.float32, "fp16": np.float16}[mode]
    n_iter = b_loc // bt
    q0 = qt_np.astype(cast_dt)
    inp_c = input_np.astype(cast_dt)
    maps = []
    for i in range(n_cores):
        a = inp_c[i * b_loc : (i + 1) * b_loc]
        # a[it*bt + b, k*128 + p] -> [p, it, k, b]
        packed = np.ascontiguousarray(
            a.reshape(n_iter, bt, KT, P).transpose(3, 0, 2, 1)
        )
        maps.append({"a0": packed, "q0": q0})
    return maps


def _unpack_out(res, n_cores, b_loc, bt=2048):
    """out_dram[p, it, s, n] -> (n_cores*b_loc, D) fp32."""
    outs = []
    for i in range(n_cores):
        o = np.asarray(res.results[i]["out"])  # [P, n_iter, sb_n, D]
        o = o.transpose(1, 2, 0, 3).reshape(b_loc, D)
        outs.append(o)
    return np.concatenate(outs, axis=0).astype(np.float32)


def _compute_qt(weight_np):
    """Q from QR(weight + 1e-8), transposed. Prefer jax-on-CPU so Q matches the
    fp32 jax reference bit-for-bit when possible; fall back to LAPACK (both are
    Householder QR and agree to ~1e-6, so either is well within tolerance)."""
    w = weight_np.astype(np.float32)
    try:
        import jax
        import jax.numpy as jnp

        cpu = jax.devices("cpu")[0]
        with jax.default_device(cpu):
            q, _ = jnp.linalg.qr(jax.device_put(w, cpu) + 1e-8)
        q = np.asarray(q)
    except Exception:
        q, _ = np.linalg.qr(w + np.float32(1e-8))
    return np.ascontiguousarray(q.T.astype(np.float32))


def run(input_np, weight_np, mode=None, n_cores=N_CORES, b_loc=None,
        layout=None, bt=None, build_kw=None, **run_kwargs):
    mode = mode or MODE
    layout = layout or LAYOUT
    bt = bt or BT_TILE
    if build_kw is None:
        build_kw = BUILD_KW
    if mode == "fp16x2":
        layout = "plain"
    b_loc = b_loc or (input_np.shape[0] // n_cores)
    assert input_np.shape[0] == n_cores * b_loc, (
        f"batch {input_np.shape[0]} not divisible into {n_cores} cores"
    )
    assert input_np.shape[1] == D

    qt = _compute_qt(weight_np)

    if layout in ("packed", "inv"):
        nc = _get_nc(mode, b_loc, layout=layout, bt=bt, **(build_kw or {}))
        in_maps = _prep_inputs_packed(
            mode, np.asarray(input_np), qt, n_cores, b_loc, bt=bt
        )
        res = run_bass_kernel_spmd(
            nc, in_maps, list(range(n_cores)), **run_kwargs
        )
        unpack = _unpack_out_inv if layout == "inv" else _unpack_out
        return unpack(res, n_cores, b_loc, bt=bt), res

    nc = _get_nc(mode, b_loc, **(build_kw or {}))
    in_maps = _prep_inputs(mode, np.asarray(input_np), qt, n_cores, b_loc)
    res = run_bass_kernel_spmd(nc, in_maps, list(range(n_cores)), **run_kwargs)
    out = np.concatenate(
        [np.asarray(res.results[i]["out"], dtype=np.float32)
         for i in range(n_cores)],
        axis=0,
    )
    return out, res


def kernel(input, weight):
    out, _ = run(
        np.asarray(input, dtype=np.float32), np.asarray(weight, dtype=np.float32)
    )
    return np.ascontiguousarray(out, dtype=np.float32)
